# revision 10
# baseline (speedup 1.0000x reference)
"""MultiHeadAttention (B=2, S=2048, D=1024, H=16) on 8 trn2 NeuronCores.

Sharding: core c -> batch b = c//4, head-group g = c%4 (4 heads each).
Each core computes its heads' q/k/v projections (Megatron column-split,
weights passed pre-transposed from host), scores+softmax (writes its 4
heads' weight matrices), ctx, and a partial output projection (row-split
Wo). Host sums the 4 partial outs per batch.

All matmuls run as float32r (1 cyc/row fp32 relaxed mode); softmax sums
come from the ScalarE activation accum_out; ctx normalization happens on
[i]-partition layout via a small PE transpose dance.
"""

import numpy as np

import concourse.bass as bass
import concourse.tile as tile
from concourse import mybir
from concourse.masks import make_identity
from concourse.vector_clock import ScopedClock

F32 = mybir.dt.float32
F32R = mybir.dt.float32r
AF = mybir.ActivationFunctionType

B, S, D, H = 2, 2048, 1024, 16
HD = D // H          # 64
HL = 4               # heads per core
JL = HL * HD         # 256 local projection cols
SCALE = 1.0 / float(np.sqrt(HD))

_PATCHED = False


def _patch_tile_drain():
    """This walrus build allows one sem-wait per instruction; TileContext's
    final drain carries one wait per DMA lane. Split them across drains."""
    global _PATCHED
    if _PATCHED:
        return
    _PATCHED = True

    def patched(self, tick_clock, wait_clock):
        d = self.nc.sync.drain()
        wait_clock.add_sem_waits(d.ins, ScopedClock({None: tick_clock.global_clock}))
        si = d.ins.sync_info
        if si is not None and si.on_wait and len(si.on_wait) > 1:
            waits = list(si.on_wait)
            si.on_wait = waits[:1]
            for w in waits[1:]:
                d2 = self.nc.sync.drain()
                i2 = d2.ins
                if i2.sync_info is None:
                    i2.sync_info = type(si)(on_wait=[w], on_update=list())
                else:
                    i2.sync_info.on_wait = [w]
        self.nc.all_engine_barrier()
        popped = self.nc._tile_sem_poison_stack.pop()
        assert popped is self._sem_poison
        self.nc.clear_and_free_semaphores(list(self.sems.allocated().values()))
        self.nc.all_engine_barrier()

    tile.TileContext._drain_and_barrier = patched


def r(ap):
    return ap.bitcast(F32R)


def _split_multiwaits(nc):
    """Walrus in this env accepts a single sem-wait per instruction; Tile
    emits several. Peel extras onto NoOps inserted just before."""
    import bass_rust

    cnt = 0
    for f in nc.m.functions:
        for bb in f.blocks:
            new_list = []
            changed = False
            for inst in bb.instructions:
                si = inst.sync_info
                if si is not None and si.on_wait and len(si.on_wait) > 1:
                    waits = list(si.on_wait)
                    for w in waits[:-1]:
                        cnt += 1
                        n = mybir.InstNoOp(name=f"I-mwsplit-{cnt}", ins=[], outs=[])
                        n.engine = inst.engine
                        n.sync_info = bass_rust.SyncInfo(on_wait=[w], on_update=[])
                        new_list.append(n)
                    si.on_wait = waits[-1:]
                    changed = True
                new_list.append(inst)
            if changed:
                bb.instructions = new_list
    return cnt


def build(s=S, phases=4):
    """Build the per-core Bass program. s = sequence length (param for tests)."""
    _patch_tile_drain()
    nc = bass.Bass("TRN2", target_bir_lowering=False, debug=False, num_devices=8)

    NI = s // 128            # i blocks
    NK = s // 128            # k blocks
    NG = s // 512            # 512-wide i groups
    DC = D // 128            # 8 contraction chunks
    JC = JL // 128           # 2 local j chunks

    xq = nc.dram_tensor("xq", [s, D], F32, kind="ExternalInput").ap()
    xk = nc.dram_tensor("xk", [s, D], F32, kind="ExternalInput").ap()
    xv = nc.dram_tensor("xv", [s, D], F32, kind="ExternalInput").ap()
    wqt = nc.dram_tensor("wqt", [D, JL], F32, kind="ExternalInput").ap()
    wkt = nc.dram_tensor("wkt", [D, JL], F32, kind="ExternalInput").ap()
    wvt = nc.dram_tensor("wvt", [D, JL], F32, kind="ExternalInput").ap()
    wot = nc.dram_tensor("wot", [JL, D], F32, kind="ExternalInput").ap()
    bq = nc.dram_tensor("bq", [JL], F32, kind="ExternalInput").ap()
    bk = nc.dram_tensor("bk", [JL], F32, kind="ExternalInput").ap()
    bv = nc.dram_tensor("bv", [JL], F32, kind="ExternalInput").ap()
    bo = nc.dram_tensor("bo", [D], F32, kind="ExternalInput").ap()
    madd = nc.dram_tensor("madd", [s], F32, kind="ExternalInput").ap()

    w_out = nc.dram_tensor("w_out", [HL, s, s], F32, kind="ExternalOutput").ap()
    out_part = nc.dram_tensor("out_part", [s, D], F32, kind="ExternalOutput").ap()

    with tile.TileContext(nc) as tc:
        import contextlib

        with contextlib.ExitStack() as ctx:
            persist = ctx.enter_context(tc.tile_pool(name="persist", bufs=1))
            ident = persist.tile([128, 128], F32)
            make_identity(nc, ident[:])

            wqT = persist.tile([128, DC * JL], F32)   # [d-part, dc-major j]
            wkT = persist.tile([128, DC * JL], F32)
            wvT = persist.tile([128, DC * JL], F32)
            woTh = [persist.tile([HD, D], F32, name=f"woTh{h}") for h in range(HL)]
            qT = persist.tile([128, JC * s], F32)     # [j-part, jc-major i]
            kT = persist.tile([128, JC * s], F32)
            v_sb = persist.tile([128, NK * JL], F32)  # [k-part, kb-major j]
            ctxTh = [persist.tile([HD, s], F32, name=f"ctxTh{h}") for h in range(HL)]

            brow_q = persist.tile([1, JL], F32)
            brow_k = persist.tile([1, JL], F32)
            brow_v = persist.tile([1, JL], F32)
            brow_o = persist.tile([1, D], F32)
            ones_row = persist.tile([1, 512], F32)
            maddT = persist.tile([128, NK], F32)

            nc.gpsimd.dma_start(maddT[:], madd.rearrange("(a b) -> b a", b=128))

            with tc.tile_pool(name="wstg", bufs=2) as wstgp:
                for brow, b_dram, bn in ((brow_q, bq, JL), (brow_k, bk, JL),
                                         (brow_v, bv, JL), (brow_o, bo, D)):
                    bstg = wstgp.tile([1, D], F32, tag="bstg")
                    nc.gpsimd.dma_start(
                        bstg[:, :bn], b_dram.rearrange("(a b) -> a b", a=1)
                    )
                    nc.vector.tensor_copy(r(brow[:]), bstg[:, :bn])
                ones_raw = wstgp.tile([1, 512], F32, tag="bstg")
                nc.vector.memset(ones_raw[:], 1.0)
                nc.vector.tensor_copy(r(ones_row[:]), ones_raw[:])
                for wT, wt_dram in ((wqT, wqt), (wkT, wkt), (wvT, wvt)):
                    wstg = wstgp.tile([128, DC * JL], F32, tag="wstg")
                    nc.gpsimd.dma_start(
                        wstg[:].rearrange("p (dc j) -> p dc j", dc=DC),
                        wt_dram.rearrange("(dc p) j -> p dc j", p=128),
                    )
                    nc.vector.tensor_copy(r(wT[:]), wstg[:])
                for h in range(HL):
                    wstg = wstgp.tile([128, DC * JL], F32, tag="wstg")
                    nc.gpsimd.dma_start(wstg[:HD, :D], wot[h * HD:(h + 1) * HD, :])
                    nc.vector.tensor_copy(r(woTh[h][:]), wstg[:HD, :D])

            # --- stage 1: transpose x, project q/k/v -----------------------
            with tc.tile_pool(name="xld", bufs=3) as xld, \
                 tc.tile_pool(name="xT", bufs=2) as xTp, \
                 tc.tile_pool(name="tps1", bufs=4, space="PSUM") as tps1, \
                 tc.tile_pool(name="pps1", bufs=2, space="PSUM") as pps1:

                def load_xT(x_dram, ig):
                    """xT tile [128 d, dc-major (8 x 512 i)] for i-group ig."""
                    xT = xTp.tile([128, DC * 512], F32, tag="xT")
                    for sub in range(4):
                        lt = xld.tile([128, D], F32, tag="xld")
                        nc.gpsimd.dma_start(
                            lt[:],
                            x_dram[ig * 512 + sub * 128: ig * 512 + (sub + 1) * 128, :],
                        )
                        for dc in range(DC):
                            tp = tps1.tile([128, 128], F32)
                            nc.tensor.transpose(
                                tp[:], lt[:, dc * 128:(dc + 1) * 128], ident[:]
                            )
                            nc.vector.tensor_copy(
                                r(xT[:, dc * 512 + sub * 128: dc * 512 + (sub + 1) * 128]),
                                tp[:],
                            )
                    return xT

                for x_dram, wT, brow, dstT in ((xq, wqT, brow_q, qT), (xk, wkT, brow_k, kT)):
                    for ig in range(NG):
                        xT = load_xT(x_dram, ig)
                        for jc in range(JC):
                            ps = pps1.tile([128, 512], F32, tag="pj")
                            for dc in range(DC):
                                nc.tensor.matmul(
                                    ps[:],
                                    r(wT[:, dc * JL + jc * 128: dc * JL + (jc + 1) * 128]),
                                    r(xT[:, dc * 512:(dc + 1) * 512]),
                                    start=(dc == 0), stop=False,
                                )
                            nc.tensor.matmul(
                                ps[:], r(brow[:, jc * 128:(jc + 1) * 128]),
                                r(ones_row[:]), start=False, stop=True,
                            )
                            nc.scalar.copy(
                                r(dstT[:, jc * s + ig * 512: jc * s + (ig + 1) * 512]),
                                ps[:],
                            )

                for ig in range(NG):
                    xT = load_xT(xv, ig)
                    for sub in range(4):
                        kb = ig * 4 + sub
                        ps = pps1.tile([128, JL], F32, tag="pv")
                        for dc in range(DC):
                            nc.tensor.matmul(
                                ps[:],
                                r(xT[:, dc * 512 + sub * 128: dc * 512 + (sub + 1) * 128]),
                                r(wvT[:, dc * JL:(dc + 1) * JL]),
                                start=(dc == 0), stop=False,
                            )
                        nc.tensor.matmul(
                            ps[:], r(ones_row[:, :128]), r(brow_v[:]),
                            start=False, stop=True,
                        )
                        nc.scalar.copy(r(v_sb[:, kb * JL:(kb + 1) * JL]), ps[:])

            # --- stage 2: per head: softmax weights + ctx ------------------
            with tc.tile_pool(name="wtile", bufs=2) as wtp, \
                 tc.tile_pool(name="seT", bufs=3) as seTp, \
                 tc.tile_pool(name="zr", bufs=4) as zrp, \
                 tc.tile_pool(name="rh", bufs=2) as rhp, \
                 tc.tile_pool(name="ctxu", bufs=1) as ctxup, \
                 tc.tile_pool(name="ctxn", bufs=3) as ctxnp, \
                 tc.tile_pool(name="outp", bufs=2) as outpp, \
                 tc.tile_pool(name="psB", bufs=2, space="PSUM") as psB, \
                 tc.tile_pool(name="psA", bufs=2, space="PSUM") as psA, \
                 tc.tile_pool(name="psC", bufs=1, space="PSUM") as psC:

                for h in range(HL if phases >= 2 else 0):
                    jcq = h // 2
                    jr = (h % 2) * HD
                    qh = qT[jr:jr + HD, jcq * s:(jcq + 1) * s]
                    kh = kT[jr:jr + HD, jcq * s:(jcq + 1) * s]
                    rH = rhp.tile([128, NI], F32)

                    # B-phase: s=[i,k], exp+sum, normalize, write weights
                    for ib in range(NI):
                        wt = wtp.tile([128, s], F32)
                        zp = zrp.tile([128, NG + 1], F32, tag="zp")
                        for kc in range(NG):
                            ps = psB.tile([128, 512], F32)
                            nc.tensor.matmul(
                                ps[:],
                                r(qh[:, ib * 128:(ib + 1) * 128]),
                                r(kh[:, kc * 512:(kc + 1) * 512]),
                                start=True, stop=True,
                            )
                            nc.scalar.activation(
                                wt[:, kc * 512:(kc + 1) * 512], ps[:],
                                AF.Exp, scale=SCALE,
                                accum_out=zp[:, kc:kc + 1],
                            )
                        nc.vector.tensor_reduce(
                            zp[:, NG:NG + 1], zp[:, :NG],
                            mybir.AxisListType.X, mybir.AluOpType.add,
                        )
                        nc.vector.reciprocal(rH[:, ib:ib + 1], zp[:, NG:NG + 1])
                        nc.vector.tensor_scalar_mul(wt[:], wt[:], rH[:, ib:ib + 1])
                        nc.gpsimd.dma_start(
                            w_out[h, ib * 128:(ib + 1) * 128, :], wt[:]
                        )

                    if phases < 3:
                        continue
                    # A-phase: sT=[k,i], exp (+mask bias), ctx accumulation
                    ctx_ps = psC.tile([HD, s], F32)
                    for ig in range(NG):
                        for kb in range(NK):
                            ps = psA.tile([128, 512], F32, tag="sT")
                            nc.tensor.matmul(
                                ps[:],
                                r(kh[:, kb * 128:(kb + 1) * 128]),
                                r(qh[:, ig * 512:(ig + 1) * 512]),
                                start=True, stop=True,
                            )
                            se = seTp.tile([128, 512], F32)
                            nc.scalar.activation(
                                r(se[:]), ps[:], AF.Exp, scale=SCALE,
                                bias=maddT[:, kb:kb + 1],
                            )
                            nc.tensor.matmul(
                                ctx_ps[:, ig * 512:(ig + 1) * 512],
                                r(v_sb[:, kb * JL + h * HD: kb * JL + (h + 1) * HD]),
                                r(se[:]),
                                start=(kb == 0), stop=(kb == NK - 1),
                            )

                    # dance: normalize ctx rows by rH on [i]-partition layout
                    cu = ctxup.tile([HD, s], F32)
                    nc.vector.tensor_copy(cu[:], ctx_ps[:])
                    for ib in range(NI):
                        tp = psA.tile([128, 512], F32, tag="sT")
                        nc.tensor.transpose(
                            tp[:, :HD], cu[:, ib * 128:(ib + 1) * 128],
                            ident[:HD, :HD],
                        )
                        cn = ctxnp.tile([128, HD], F32)
                        nc.vector.tensor_scalar_mul(cn[:], tp[:, :HD], rH[:, ib:ib + 1])
                        tp2 = psA.tile([128, 512], F32, tag="sT")
                        nc.tensor.transpose(tp2[:HD, :128], cn[:], ident[:])
                        nc.scalar.copy(
                            r(ctxTh[h][:, ib * 128:(ib + 1) * 128]), tp2[:HD, :128]
                        )

                # --- stage 3: partial out projection -----------------------
                for ib in range(NI if phases >= 4 else 0):
                    ot = outpp.tile([128, D], F32)
                    for oc in range(2):
                        ps = psB.tile([128, 512], F32)
                        for h in range(HL):
                            nc.tensor.matmul(
                                ps[:],
                                r(ctxTh[h][:, ib * 128:(ib + 1) * 128]),
                                r(woTh[h][:, oc * 512:(oc + 1) * 512]),
                                start=(h == 0), stop=False,
                            )
                        nc.tensor.matmul(
                            ps[:], r(ones_row[:, :128]),
                            r(brow_o[:, oc * 512:(oc + 1) * 512]),
                            start=False, stop=True,
                        )
                        nc.scalar.copy(ot[:, oc * 512:(oc + 1) * 512], ps[:])
                    nc.gpsimd.dma_start(out_part[ib * 128:(ib + 1) * 128, :], ot[:])

    _split_multiwaits(nc)
    return nc


_NC_CACHE = {}


def _get_nc(s=S, phases=4):
    key = (s, phases)
    if key not in _NC_CACHE:
        _NC_CACHE[key] = build(s, phases)
    return _NC_CACHE[key]


def _reference_np(query, key, value, mask, Wq, bq, Wk, bk, Wv, bv, Wo, bo):
    """Numpy fallback (only used for masks with zeros)."""
    b = query.shape[0]

    def split(x):
        return x.reshape(b, -1, H, HD).transpose(0, 2, 1, 3)

    q = split(query @ Wq.T + bq)
    k = split(key @ Wk.T + bk)
    v = split(value @ Wv.T + bv)
    scores = np.einsum("bhqd,bhkd->bhqk", q, k) / np.float32(np.sqrt(HD))
    scores = np.where(mask == 0, np.float32(-1e10), scores)
    scores -= scores.max(axis=-1, keepdims=True)
    e = np.exp(scores)
    weights = e / e.sum(axis=-1, keepdims=True)
    ctx = np.einsum("bhqk,bhkd->bhqd", weights, v)
    ctx = ctx.transpose(0, 2, 1, 3).reshape(b, -1, D)
    out = ctx @ Wo.T + bo
    return out.astype(np.float32), weights.astype(np.float32)


def kernel(query, key, value, mask, Wq, bq, Wk, bk, Wv, bv, Wo, bo,
           _trace=False, _s=S):
    query = np.ascontiguousarray(np.asarray(query, np.float32))
    key = np.ascontiguousarray(np.asarray(key, np.float32))
    value = np.ascontiguousarray(np.asarray(value, np.float32))
    mask = np.asarray(mask)
    Wq, Wk, Wv, Wo = (np.ascontiguousarray(np.asarray(w, np.float32))
                      for w in (Wq, Wk, Wv, Wo))
    bq, bk, bv, bo = (np.ascontiguousarray(np.asarray(x, np.float32))
                      for x in (bq, bk, bv, bo))

    if np.any(mask == 0):
        return _reference_np(query, key, value, mask, Wq, bq, Wk, bk,
                             Wv, bv, Wo, bo)

    s = _s
    nc = _get_nc(s)
    madd_b = [np.zeros((s,), np.float32) for _ in range(B)]

    in_maps = []
    for c in range(8):
        b, g = c // 4, c % 4
        sl = slice(g * JL, (g + 1) * JL)
        in_maps.append({
            "xq": query[b], "xk": key[b], "xv": value[b],
            "wqt": np.ascontiguousarray(Wq[sl].T),
            "wkt": np.ascontiguousarray(Wk[sl].T),
            "wvt": np.ascontiguousarray(Wv[sl].T),
            "wot": np.ascontiguousarray(Wo[:, sl].T),
            "bq": np.ascontiguousarray(bq[sl]),
            "bk": np.ascontiguousarray(bk[sl]),
            "bv": np.ascontiguousarray(bv[sl]),
            "bo": bo,
            "madd": madd_b[b],
        })

    from concourse.bass_utils import run_bass_kernel_spmd
    res = run_bass_kernel_spmd(nc, in_maps, list(range(8)), trace=_trace)

    out = np.zeros((B, s, D), np.float32)
    weights = np.empty((B, H, s, s), np.float32)
    for c in range(8):
        b, g = c // 4, c % 4
        out[b] += res.results[c]["out_part"]
        weights[b, g * HL:(g + 1) * HL] = res.results[c]["w_out"]

    if _trace:
        kernel._last_results = res
    return out, weights
